# revision 9
# baseline (speedup 1.0000x reference)
"""VGAE (2-layer GCN encoder + inner-product decoder) on 8 trn2 NeuronCores.

Sharding: 1D node partitioning. Core d owns output rows I_d = [d*1024, (d+1)*1024).
Inputs per core: adjT shard adj[I_d,:].T (so matmul contraction runs along
partitions with no on-device transposes), xT column slice, replicated weights.

All heavy matmuls run as fp32r (PE full-rate for moving-free >= 256, ~12-bit
mantissa rounding, rel err ~1e-4 vs fp32).

Pipeline per core:
  A: S1_local = (x @ W1)[I_d]            -> AllGather -> S1 [8192, 64]
  B: h1T = relu(S1.T-contract adjT)      [64, 1024]   (64 k-tile psum accum)
  C: S2_local = h1 @ [W2|W3]             -> AllGather -> S2 [8192, 128]
  D: [muT; logvarT] = S2-contract adjT   [128, 1024]
  E: zT AllGather; recon rows = zT_loc.T @ zT_full -> [1024, 8192]
"""

import numpy as np

N = 8192
NFEAT = 256
NHID = 64
NC = 8
SH = N // NC          # 1024 rows per core
KT = N // 128         # 64 contraction k-tiles
CH = 256              # matmul moving-free chunk (fp32r full-rate minimum)
NCH = SH // CH        # 4 chunks across the local 1024 columns
RES = 12              # adjT k-tiles kept SBUF-resident between layer 1 and 2

_cache = {}


def _build_nc():
    import concourse.tile as tile
    from concourse import bacc, mybir

    F32 = mybir.dt.float32
    F32R = mybir.dt.float32r
    Relu = mybir.ActivationFunctionType.Relu
    Copy = mybir.ActivationFunctionType.Copy

    nc = bacc.Bacc("TRN2", target_bir_lowering=False, debug=False, num_devices=NC)

    adjT = nc.dram_tensor("adjT", [N, SH], F32, kind="ExternalInput").ap()
    xts = nc.dram_tensor("xTs", [NFEAT, SH], F32, kind="ExternalInput").ap()
    w1 = nc.dram_tensor("W1", [NFEAT, NHID], F32, kind="ExternalInput").ap()
    w23 = nc.dram_tensor("W23", [NHID, 2 * NHID], F32, kind="ExternalInput").ap()
    recon_out = nc.dram_tensor("recon_rows", [SH, N], F32, kind="ExternalOutput").ap()
    muT_out = nc.dram_tensor("muT_part", [NHID, SH], F32, kind="ExternalOutput").ap()
    lvT_out = nc.dram_tensor("logvarT_part", [NHID, SH], F32, kind="ExternalOutput").ap()

    with tile.TileContext(nc) as tc:
        with (
            tc.tile_pool(name="persist", bufs=1) as pp,
            tc.tile_pool(name="adjres", bufs=1) as arp,
            tc.tile_pool(name="adjstream", bufs=6) as asp,
            tc.tile_pool(name="recstage", bufs=2) as rsp,
            tc.tile_pool(name="psum", bufs=4, space="PSUM") as pipe,
            tc.tile_pool(name="psacc", bufs=4, space="PSUM") as pacc,
            tc.tile_pool(name="dram", bufs=1, space="DRAM") as dp,
        ):
            rg = [list(range(NC))]

            # ---- constants / persistent sbuf ----
            w1_sb = pp.tile([128, 2 * NHID], F32R, tag="w1")
            for jt in range(2):
                nc.gpsimd.dma_start(
                    w1_sb[:, jt * NHID:(jt + 1) * NHID], w1[jt * 128:(jt + 1) * 128, :]
                )
            w23_sb = pp.tile([NHID, 2 * NHID], F32R, tag="w23")
            nc.gpsimd.dma_start(w23_sb[:], w23[:])

            s1_sb = pp.tile([128, KT * NHID], F32R, tag="s1")       # S1, lhsT layout
            h1t_sb = pp.tile([NHID, SH], F32R, tag="h1t")           # relu(adj@S1).T local
            s2_sb = pp.tile([128, KT * 128], F32R, tag="s2")        # S2 full, lhsT layout
            zt_sb = pp.tile([NHID, SH], F32R, tag="zt")             # muT local
            lvt_sb = pp.tile([NHID, SH], F32, tag="lvt")            # logvarT local
            ztf_sb = pp.tile([NHID, N], F32R, tag="ztf")            # zT full

            # ---- phase A: S1_local = (x @ W1)[I_d], AllGather to S1 full ----
            xts_sb = pp.tile([128, 2 * SH], F32R, tag="xts")
            for jt in range(2):
                nc.gpsimd.dma_start(
                    xts_sb[:, jt * SH:(jt + 1) * SH], xts[jt * 128:(jt + 1) * 128, :]
                )
            s1l_sb = pp.tile([128, (SH // 128) * NHID], F32R, tag="s1l")
            for kt in range(SH // 128):
                ps = pipe.tile([128, NHID], F32, tag="pipe")
                for jt in range(2):
                    nc.tensor.matmul(
                        ps[:],
                        xts_sb[:, jt * SH + kt * 128: jt * SH + (kt + 1) * 128],
                        w1_sb[:, jt * NHID:(jt + 1) * NHID],
                        start=(jt == 0),
                        stop=(jt == 1),
                    )
                nc.vector.tensor_copy(s1l_sb[:, kt * NHID:(kt + 1) * NHID], ps[:])
            s1in = dp.tile([SH, NHID], F32R, tag="s1in")
            for kt in range(SH // 128):
                nc.sync.dma_start(
                    s1in[kt * 128:(kt + 1) * 128, :],
                    s1l_sb[:, kt * NHID:(kt + 1) * NHID],
                )
            s1g = dp.tile([N, NHID], F32R, tag="s1g", addr_space="Shared")
            nc.gpsimd.collective_compute(
                "AllGather", mybir.AluOpType.bypass,
                replica_groups=rg, ins=[s1in.opt()], outs=[s1g.opt()],
            )
            for g in range(NC):
                nc.sync.dma_start(
                    s1_sb[
                        :, g * (KT // NC) * NHID:(g + 1) * (KT // NC) * NHID
                    ].rearrange("p (kt f) -> p kt f", f=NHID),
                    s1g[g * SH:(g + 1) * SH, :].rearrange("(kt p) f -> p kt f", p=128),
                )

            # ---- phase B: h1T = relu( sum_k S1[k,:]^T adjT[k, :] ) ----
            adj_res = []
            for kt in range(RES):
                at = arp.tile([128, SH], F32R, tag=f"ar{kt}")
                nc.gpsimd.dma_start(at[:], adjT[kt * 128:(kt + 1) * 128, :])
                adj_res.append(at)

            # One PSUM bank per accumulation slice: start=True clears the
            # has_written bits of the whole bank, so concurrent accumulation
            # groups must not share a bank.
            hps = [pacc.tile([NHID, CH], F32, tag="acc", name=f"hps{c}") for c in range(NCH)]
            for kt in range(KT):
                if kt < RES:
                    at = adj_res[kt]
                else:
                    at = asp.tile([128, SH], F32R, tag="adj")
                    nc.gpsimd.dma_start(at[:], adjT[kt * 128:(kt + 1) * 128, :])
                for c in range(NCH):
                    nc.tensor.matmul(
                        hps[c][:],
                        s1_sb[:, kt * NHID:(kt + 1) * NHID],
                        at[:, c * CH:(c + 1) * CH],
                        start=(kt == 0),
                        stop=(kt == KT - 1),
                    )
            for c in range(NCH):
                nc.scalar.activation(
                    h1t_sb[:, c * CH:(c + 1) * CH], hps[c][:], Relu
                )

            # ---- phase C: S2_local = h1 @ [W2|W3], AllGather to S2 full ----
            s2l_sb = pp.tile([128, (SH // 128) * 128], F32R, tag="s2l")
            for kt in range(SH // 128):
                ps = pipe.tile([128, 128], F32, tag="pipe")
                nc.tensor.matmul(
                    ps[:],
                    h1t_sb[:, kt * 128:(kt + 1) * 128],
                    w23_sb[:],
                    start=True,
                    stop=True,
                )
                nc.vector.tensor_copy(s2l_sb[:, kt * 128:(kt + 1) * 128], ps[:])
            s2in = dp.tile([SH, 128], F32R, tag="s2in")
            for kt in range(SH // 128):
                nc.sync.dma_start(
                    s2in[kt * 128:(kt + 1) * 128, :],
                    s2l_sb[:, kt * 128:(kt + 1) * 128],
                )
            s2g = dp.tile([N, 128], F32R, tag="s2g", addr_space="Shared")
            nc.gpsimd.collective_compute(
                "AllGather", mybir.AluOpType.bypass,
                replica_groups=rg, ins=[s2in.opt()], outs=[s2g.opt()],
            )
            for g in range(NC):
                nc.sync.dma_start(
                    s2_sb[
                        :, g * (KT // NC) * 128:(g + 1) * (KT // NC) * 128
                    ].rearrange("p (kt f) -> p kt f", f=128),
                    s2g[g * SH:(g + 1) * SH, :].rearrange("(kt p) f -> p kt f", p=128),
                )

            # ---- phase D: [muT; logvarT] = sum_k S2[k,:]^T adjT[k,:] ----
            mlps = [pacc.tile([128, CH], F32, tag="acc", name=f"mlps{c}") for c in range(NCH)]
            for kt in range(KT):
                if kt < RES:
                    at = adj_res[kt]
                else:
                    at = asp.tile([128, SH], F32R, tag="adj")
                    nc.gpsimd.dma_start(at[:], adjT[kt * 128:(kt + 1) * 128, :])
                for c in range(NCH):
                    nc.tensor.matmul(
                        mlps[c][:],
                        s2_sb[:, kt * 128:(kt + 1) * 128],
                        at[:, c * CH:(c + 1) * CH],
                        start=(kt == 0),
                        stop=(kt == KT - 1),
                    )
            for c in range(NCH):
                nc.vector.tensor_copy(
                    zt_sb[:, c * CH:(c + 1) * CH], mlps[c][0:NHID, :]
                )
                nc.scalar.activation(
                    lvt_sb[:, c * CH:(c + 1) * CH],
                    mlps[c][NHID:128, :],
                    Copy,
                )
            nc.sync.dma_start(muT_out[:], zt_sb[:].bitcast(F32))
            nc.sync.dma_start(lvT_out[:], lvt_sb[:])

            # zT AllGather
            zin = dp.tile([NHID, SH], F32R, tag="zin")
            nc.sync.dma_start(zin[:], zt_sb[:])
            zg = dp.tile([NC * NHID, SH], F32R, tag="zg", addr_space="Shared")
            nc.gpsimd.collective_compute(
                "AllGather", mybir.AluOpType.bypass,
                replica_groups=rg, ins=[zin.opt()], outs=[zg.opt()],
            )
            for g in range(NC):
                nc.sync.dma_start(
                    ztf_sb[:, g * SH:(g + 1) * SH],
                    zg[g * NHID:(g + 1) * NHID, :],
                )

            # ---- phase E: recon rows = zT_loc.T @ zT_full ----
            NGR = 8                      # psum chunks per staging tile
            WID = NGR * CH               # 2048 cols per output DMA
            for it in range(SH // 128):
                for cg in range(N // WID):
                    rec = rsp.tile([128, WID], F32, tag="rec")
                    for cc in range(NGR):
                        c = cg * NGR + cc
                        rp = pipe.tile([128, CH], F32, tag="pipe")
                        nc.tensor.matmul(
                            rp[:],
                            zt_sb[:, it * 128:(it + 1) * 128],
                            ztf_sb[:, c * CH:(c + 1) * CH],
                            start=True,
                            stop=True,
                        )
                        if cc % 2 == 0:
                            nc.vector.tensor_copy(rec[:, cc * CH:(cc + 1) * CH], rp[:])
                        else:
                            nc.scalar.activation(
                                rec[:, cc * CH:(cc + 1) * CH], rp[:], Copy
                            )
                    nc.sync.dma_start(
                        recon_out[it * 128:(it + 1) * 128, cg * WID:(cg + 1) * WID],
                        rec[:],
                    )

    nc.compile()
    return nc


def _get_nc():
    if "nc" not in _cache:
        _cache["nc"] = _build_nc()
    return _cache["nc"]


def run_sharded(x, adj, W1, W2, W3, trace=False, tmpdir=None):
    from concourse.bass_utils import run_bass_kernel_spmd

    nc = _get_nc()
    xT = np.ascontiguousarray(x.T)
    W23 = np.ascontiguousarray(np.concatenate([W2, W3], axis=1))
    in_maps = []
    for d in range(NC):
        in_maps.append(
            {
                "adjT": np.ascontiguousarray(adj[d * SH:(d + 1) * SH, :].T),
                "xTs": np.ascontiguousarray(xT[:, d * SH:(d + 1) * SH]),
                "W1": W1,
                "W23": W23,
            }
        )
    return run_bass_kernel_spmd(
        nc, in_maps, list(range(NC)), trace=trace, tmpdir=tmpdir
    )


def kernel(x, adj, W1, W2, W3):
    br = run_sharded(x, adj, W1, W2, W3)
    recon = np.concatenate([br.results[d]["recon_rows"] for d in range(NC)], axis=0)
    mu = np.concatenate(
        [br.results[d]["muT_part"].T for d in range(NC)], axis=0
    )
    logvar = np.concatenate(
        [br.results[d]["logvarT_part"].T for d in range(NC)], axis=0
    )
    return recon, mu, logvar


# revision 11
# speedup vs baseline: 1.3877x; 1.3877x over previous
"""VGAE (2-layer GCN encoder + inner-product decoder) on 8 trn2 NeuronCores.

Sharding: 1D node partitioning. Core d owns output rows I_d = [d*1024, (d+1)*1024).
Per-core inputs: adjT shard adj[I_d,:].T in fp16 (matmul contraction runs along
partitions, no on-device transposes), full xT in fp16, replicated weights fp16.

All matmuls run in fp16 (PE full rate, 1 cycle/row; fp32 PSUM accumulate).
fp16's 11-bit mantissa gives ~2.4e-4 relative rounding — validated end-to-end
~4e-4 vs the fp32 reference. mu peaks at ~283k > fp16 max, so the decoder
uses z/16 (exact power-of-two scale) and rescales recon by 256 at PSUM
evacuation.

Engine/queue plan:
  sync (HWDGE):   streaming loads (adjT, xT, weights) - never blocked
  gpsimd (SWDGE): collective bounces, load-backs, all stores
  vector:         collective trigger/wait (+ psum evacuation copies)
  scalar:         psum evacuation copies / relu / scaling
  52 of 64 adjT k-tiles stay SBUF-resident between layer 1 and layer 2.

Pipeline per core:
  A: S1 = x @ W1 (full, fp16)                      [8192, 64]
  B: h1T = relu(S1.T-contract adjT)                [64, 1024]
  C: S2_local = h1 @ [W2|W3] -> AllGather -> S2    [8192, 128]
  D: [muT; logvarT] = S2-contract adjT             [128, 1024]
  E: zT = muT/16 AllGather; recon = 256*(zT_loc.T @ zT_full) -> [1024, 8192]
"""

import numpy as np

N = 8192
NFEAT = 256
NHID = 64
NC = 8
SH = N // NC          # 1024 rows per core
KT = N // 128         # 64 contraction k-tiles
CH = 512              # matmul moving-free chunk (one fp32 PSUM bank)
NCH = SH // CH        # 2 chunks across the local 1024 columns
RES = 52              # adjT k-tiles kept SBUF-resident between layer 1 and 2
ZSCALE = 16.0         # z = mu / ZSCALE to keep the decoder inside fp16 range

_cache = {}


def _build_nc():
    import concourse.tile as tile
    from concourse import bacc, mybir

    F32 = mybir.dt.float32
    F16 = mybir.dt.float16
    Relu = mybir.ActivationFunctionType.Relu
    Copy = mybir.ActivationFunctionType.Copy

    nc = bacc.Bacc("TRN2", target_bir_lowering=False, debug=False, num_devices=NC)

    adjT = nc.dram_tensor("adjT", [N, SH], F16, kind="ExternalInput").ap()
    xT = nc.dram_tensor("xT", [NFEAT, N], F16, kind="ExternalInput").ap()
    w1 = nc.dram_tensor("W1", [NFEAT, NHID], F16, kind="ExternalInput").ap()
    w23 = nc.dram_tensor("W23", [NHID, 2 * NHID], F16, kind="ExternalInput").ap()
    recon_out = nc.dram_tensor("recon_rows", [SH, N], F32, kind="ExternalOutput").ap()
    muT_out = nc.dram_tensor("muT_part", [NHID, SH], F32, kind="ExternalOutput").ap()
    lvT_out = nc.dram_tensor("logvarT_part", [NHID, SH], F32, kind="ExternalOutput").ap()

    with tile.TileContext(nc) as tc:
        with (
            tc.tile_pool(name="persist", bufs=1) as pp,
            tc.tile_pool(name="adjres", bufs=1) as arp,
            tc.tile_pool(name="adjstream", bufs=6) as asp,
            tc.tile_pool(name="xstream", bufs=4) as xsp,
            tc.tile_pool(name="recstage", bufs=2) as rsp,
            tc.tile_pool(name="psum", bufs=5, space="PSUM") as pipe,
            tc.tile_pool(name="psacc", bufs=2, space="PSUM") as pacc,
            tc.tile_pool(name="dram", bufs=1, space="DRAM") as dp,
        ):
            rg = [list(range(NC))]

            # ---- weights ----
            w1_sb = pp.tile([128, 2 * NHID], F16, tag="w1")
            for jt in range(2):
                nc.sync.dma_start(
                    w1_sb[:, jt * NHID:(jt + 1) * NHID], w1[jt * 128:(jt + 1) * 128, :]
                )
            w23_sb = pp.tile([NHID, 2 * NHID], F16, tag="w23")
            nc.sync.dma_start(w23_sb[:], w23[:])

            # ---- persistent sbuf ----
            s1_sb = pp.tile([128, KT * NHID], F16, tag="s1")        # S1, lhsT layout
            h1t_sb = pp.tile([NHID, SH], F16, tag="h1t")            # relu(adj@S1).T local
            s2_sb = pp.tile([128, KT * 128], F16, tag="s2")         # S2 full, lhsT layout
            zt_sb = pp.tile([NHID, SH], F16, tag="zt")              # muT/16 local (decoder)
            mut_sb = pp.tile([NHID, SH], F32, tag="mut")            # muT local, f32 out
            lvt_sb = pp.tile([NHID, SH], F32, tag="lvt")            # logvarT local, f32 out
            ztf_sb = pp.tile([NHID, N], F16, tag="ztf")             # zT full

            # ---- resident adjT k-tiles (loaded once, used by both layers) ----
            adj_res = []
            for kt in range(RES):
                at = arp.tile([128, SH], F16, tag=f"ar{kt}", name=f"ar{kt}")
                nc.sync.dma_start(at[:], adjT[kt * 128:(kt + 1) * 128, :])
                adj_res.append(at)

            # ---- phase A: S1 = x @ W1 (computed fully on every core) ----
            for g in range(NC):
                xt0 = xsp.tile([128, SH], F16, tag="xt0")
                nc.sync.dma_start(xt0[:], xT[0:128, g * SH:(g + 1) * SH])
                xt1 = xsp.tile([128, SH], F16, tag="xt1")
                nc.sync.dma_start(xt1[:], xT[128:256, g * SH:(g + 1) * SH])
                for kk in range(SH // 128):
                    kt = g * (SH // 128) + kk
                    ps = pipe.tile([128, NHID], F32, tag="pipe")
                    nc.tensor.matmul(
                        ps[:], xt0[:, kk * 128:(kk + 1) * 128],
                        w1_sb[:, 0:NHID], start=True, stop=False,
                    )
                    nc.tensor.matmul(
                        ps[:], xt1[:, kk * 128:(kk + 1) * 128],
                        w1_sb[:, NHID:2 * NHID], start=False, stop=True,
                    )
                    nc.vector.tensor_copy(s1_sb[:, kt * NHID:(kt + 1) * NHID], ps[:])

            # ---- phase B: h1T = relu( sum_k S1[k,:]^T adjT[k,:] ) ----
            # one PSUM bank per accumulation slice (start=True clears whole bank)
            hps = [
                pacc.tile([NHID, CH], F32, tag="acc", name=f"hps{c}")
                for c in range(NCH)
            ]
            for kt in range(KT):
                if kt < RES:
                    at = adj_res[kt]
                else:
                    at = asp.tile([128, SH], F16, tag="adj")
                    nc.sync.dma_start(at[:], adjT[kt * 128:(kt + 1) * 128, :])
                for c in range(NCH):
                    nc.tensor.matmul(
                        hps[c][:],
                        s1_sb[:, kt * NHID:(kt + 1) * NHID],
                        at[:, c * CH:(c + 1) * CH],
                        start=(kt == 0),
                        stop=(kt == KT - 1),
                    )
            for c in range(NCH):
                nc.scalar.activation(h1t_sb[:, c * CH:(c + 1) * CH], hps[c][:], Relu)

            # ---- phase C: S2_local = h1 @ [W2|W3], AllGather ----
            s2l_sb = pp.tile([128, (SH // 128) * 128], F16, tag="s2l")
            for kk in range(SH // 128):
                ps = pipe.tile([128, 128], F32, tag="pipe")
                nc.tensor.matmul(
                    ps[:], h1t_sb[:, kk * 128:(kk + 1) * 128], w23_sb[:],
                    start=True, stop=True,
                )
                nc.vector.tensor_copy(s2l_sb[:, kk * 128:(kk + 1) * 128], ps[:])
            s2in = dp.tile([SH, 128], F16, tag="s2in")
            for kk in range(SH // 128):
                nc.gpsimd.dma_start(
                    s2in[kk * 128:(kk + 1) * 128, :], s2l_sb[:, kk * 128:(kk + 1) * 128]
                )
            s2g = dp.tile([N, 128], F16, tag="s2g", addr_space="Shared")
            nc.gpsimd.collective_compute(
                "AllGather", mybir.AluOpType.bypass,
                replica_groups=rg, ins=[s2in.opt()], outs=[s2g.opt()],
            )
            for g in range(NC):
                nc.gpsimd.dma_start(
                    s2_sb[
                        :, g * (KT // NC) * 128:(g + 1) * (KT // NC) * 128
                    ].rearrange("p (kt f) -> p kt f", f=128),
                    s2g[g * SH:(g + 1) * SH, :].rearrange("(kt p) f -> p kt f", p=128),
                )

            # ---- phase D: [muT; logvarT] = sum_k S2[k,:]^T adjT[k,:] ----
            mlps = [
                pacc.tile([128, CH], F32, tag="acc", name=f"mlps{c}")
                for c in range(NCH)
            ]
            for kt in range(KT):
                if kt < RES:
                    at = adj_res[kt]
                else:
                    at = asp.tile([128, SH], F16, tag="adj")
                    nc.sync.dma_start(at[:], adjT[kt * 128:(kt + 1) * 128, :])
                for c in range(NCH):
                    nc.tensor.matmul(
                        mlps[c][:],
                        s2_sb[:, kt * 128:(kt + 1) * 128],
                        at[:, c * CH:(c + 1) * CH],
                        start=(kt == 0),
                        stop=(kt == KT - 1),
                    )
            for c in range(NCH):
                sl = slice(c * CH, (c + 1) * CH)
                nc.vector.tensor_copy(mut_sb[:, sl], mlps[c][0:NHID, :])
                nc.scalar.activation(lvt_sb[:, sl], mlps[c][NHID:128, :], Copy)
                nc.scalar.activation(
                    zt_sb[:, sl], mlps[c][0:NHID, :], Copy, scale=1.0 / ZSCALE
                )
            nc.gpsimd.dma_start(muT_out[:], mut_sb[:])
            nc.gpsimd.dma_start(lvT_out[:], lvt_sb[:])

            # zT AllGather
            zin = dp.tile([NHID, SH], F16, tag="zin")
            nc.gpsimd.dma_start(zin[:], zt_sb[:])
            zg = dp.tile([NC * NHID, SH], F16, tag="zg", addr_space="Shared")
            nc.gpsimd.collective_compute(
                "AllGather", mybir.AluOpType.bypass,
                replica_groups=rg, ins=[zin.opt()], outs=[zg.opt()],
            )
            for g in range(NC):
                nc.gpsimd.dma_start(
                    ztf_sb[:, g * SH:(g + 1) * SH], zg[g * NHID:(g + 1) * NHID, :]
                )

            # ---- phase E: recon rows = 256 * (zT_loc.T @ zT_full) ----
            NGR = 4                       # psum chunks per staging tile
            WID = NGR * CH                # 2048 cols per output DMA
            for it in range(SH // 128):
                for cg in range(N // WID):
                    rec = rsp.tile([128, WID], F32, tag="rec")
                    for cc in range(NGR):
                        c = cg * NGR + cc
                        rp = pipe.tile([128, CH], F32, tag="pipe")
                        nc.tensor.matmul(
                            rp[:],
                            zt_sb[:, it * 128:(it + 1) * 128],
                            ztf_sb[:, c * CH:(c + 1) * CH],
                            start=True,
                            stop=True,
                        )
                        sl = slice(cc * CH, (cc + 1) * CH)
                        if cc % 2 == 0:
                            nc.vector.tensor_scalar_mul(
                                rec[:, sl], rp[:], ZSCALE * ZSCALE
                            )
                        else:
                            nc.scalar.activation(
                                rec[:, sl], rp[:], Copy, scale=ZSCALE * ZSCALE
                            )
                    nc.gpsimd.dma_start(
                        recon_out[it * 128:(it + 1) * 128, cg * WID:(cg + 1) * WID],
                        rec[:],
                    )

    nc.compile()
    return nc


def _get_nc():
    if "nc" not in _cache:
        _cache["nc"] = _build_nc()
    return _cache["nc"]


def run_sharded(x, adj, W1, W2, W3, trace=False, tmpdir=None):
    from concourse.bass_utils import run_bass_kernel_spmd

    nc = _get_nc()
    xT16 = np.ascontiguousarray(x.T).astype(np.float16)
    W116 = W1.astype(np.float16)
    W23 = np.concatenate([W2, W3], axis=1).astype(np.float16)
    in_maps = []
    for d in range(NC):
        in_maps.append(
            {
                "adjT": np.ascontiguousarray(adj[d * SH:(d + 1) * SH, :].T).astype(
                    np.float16
                ),
                "xT": xT16,
                "W1": W116,
                "W23": W23,
            }
        )
    return run_bass_kernel_spmd(
        nc, in_maps, list(range(NC)), trace=trace, tmpdir=tmpdir
    )


def kernel(x, adj, W1, W2, W3):
    br = run_sharded(x, adj, W1, W2, W3)
    recon = np.concatenate([br.results[d]["recon_rows"] for d in range(NC)], axis=0)
    mu = np.concatenate([br.results[d]["muT_part"].T for d in range(NC)], axis=0)
    logvar = np.concatenate(
        [br.results[d]["logvarT_part"].T for d in range(NC)], axis=0
    )
    return recon, mu, logvar


# revision 12
# speedup vs baseline: 1.6021x; 1.1545x over previous
"""VGAE (2-layer GCN encoder + inner-product decoder) on 8 trn2 NeuronCores.

Sharding: 1D node partitioning. Core d owns output rows I_d = [d*1024, (d+1)*1024).
Per-core inputs: adjT shard adj[I_d,:].T in fp16 (matmul contraction runs along
partitions, no on-device transposes), full xT in fp16, replicated weights fp16.

All matmuls run in fp16 (PE full rate, 1 cycle/row; fp32 PSUM accumulate).
fp16's 11-bit mantissa gives ~2.4e-4 relative rounding — validated end-to-end
~4e-4 vs the fp32 reference. mu peaks at ~283k > fp16 max, so the decoder
uses z/16 (exact power-of-two scale) and rescales recon by 256 at PSUM
evacuation.

Engine/queue plan:
  sync (HWDGE):   streaming loads (adjT, xT, weights) - never blocked
  gpsimd (SWDGE): collective bounces, load-backs, all stores
  vector:         collective trigger/wait (+ psum evacuation copies)
  scalar:         psum evacuation copies / relu / scaling
  52 of 64 adjT k-tiles stay SBUF-resident between layer 1 and layer 2.

Pipeline per core:
  A: S1 = x @ W1 (full, fp16)                      [8192, 64]
  B: h1T = relu(S1.T-contract adjT)                [64, 1024]
  C: S2_local = h1 @ [W2|W3] -> AllGather -> S2    [8192, 128]
  D: [muT; logvarT] = S2-contract adjT             [128, 1024]
  E: zT = muT/16 AllGather; recon = 256*(zT_loc.T @ zT_full) -> [1024, 8192]
"""

import numpy as np

N = 8192
NFEAT = 256
NHID = 64
NC = 8
SH = N // NC          # 1024 rows per core
KT = N // 128         # 64 contraction k-tiles
CH = 512              # matmul moving-free chunk (one fp32 PSUM bank)
NCH = SH // CH        # 2 chunks across the local 1024 columns
RES = 48              # adjT k-tiles kept SBUF-resident between layer 1 and 2
ZSCALE = 16.0         # z = mu / ZSCALE to keep the decoder inside fp16 range

_cache = {}


def _build_nc():
    import concourse.tile as tile
    from concourse import bacc, mybir

    F32 = mybir.dt.float32
    F16 = mybir.dt.float16
    Relu = mybir.ActivationFunctionType.Relu
    Copy = mybir.ActivationFunctionType.Copy

    nc = bacc.Bacc("TRN2", target_bir_lowering=False, debug=False, num_devices=NC)

    adjT = nc.dram_tensor("adjT", [N, SH], F16, kind="ExternalInput").ap()
    xT = nc.dram_tensor("xT", [NFEAT, N], F16, kind="ExternalInput").ap()
    w1 = nc.dram_tensor("W1", [NFEAT, NHID], F16, kind="ExternalInput").ap()
    w23 = nc.dram_tensor("W23", [NHID, 2 * NHID], F16, kind="ExternalInput").ap()
    recon_out = nc.dram_tensor("recon_rows", [SH, N], F32, kind="ExternalOutput").ap()
    muT_out = nc.dram_tensor("muT_part", [NHID, SH], F32, kind="ExternalOutput").ap()
    lvT_out = nc.dram_tensor("logvarT_part", [NHID, SH], F32, kind="ExternalOutput").ap()

    with tile.TileContext(nc) as tc:
        with (
            tc.tile_pool(name="persist", bufs=1) as pp,
            tc.tile_pool(name="adjres", bufs=1) as arp,
            tc.tile_pool(name="adjstream", bufs=8) as asp,
            tc.tile_pool(name="xstream", bufs=4) as xsp,
            tc.tile_pool(name="recstage", bufs=3) as rsp,
            tc.tile_pool(name="psum", bufs=5, space="PSUM") as pipe,
            tc.tile_pool(name="psacc", bufs=2, space="PSUM") as pacc,
            tc.tile_pool(name="dram", bufs=1, space="DRAM") as dp,
        ):
            rg = [list(range(NC))]

            # ---- weights ----
            w1_sb = pp.tile([128, 2 * NHID], F16, tag="w1")
            for jt in range(2):
                nc.sync.dma_start(
                    w1_sb[:, jt * NHID:(jt + 1) * NHID], w1[jt * 128:(jt + 1) * 128, :]
                )
            w23_sb = pp.tile([NHID, 2 * NHID], F16, tag="w23")
            nc.sync.dma_start(w23_sb[:], w23[:])

            # ---- persistent sbuf ----
            s1_sb = pp.tile([128, KT * NHID], F16, tag="s1")        # S1, lhsT layout
            h1t_sb = pp.tile([NHID, SH], F16, tag="h1t")            # relu(adj@S1).T local
            s2_sb = pp.tile([128, KT * 128], F16, tag="s2")         # S2 full, lhsT layout
            zt_sb = pp.tile([NHID, SH], F16, tag="zt")              # muT/16 local (decoder)
            mut_sb = pp.tile([NHID, SH], F32, tag="mut")            # muT local, f32 out
            lvt_sb = pp.tile([NHID, SH], F32, tag="lvt")            # logvarT local, f32 out
            ztf_sb = pp.tile([NHID, N], F16, tag="ztf")             # zT full

            # ---- phase A: S1 = x @ W1 (computed fully on every core) ----
            for g in range(NC):
                xt0 = xsp.tile([128, SH], F16, tag="xt0")
                nc.sync.dma_start(xt0[:], xT[0:128, g * SH:(g + 1) * SH])
                xt1 = xsp.tile([128, SH], F16, tag="xt1")
                nc.sync.dma_start(xt1[:], xT[128:256, g * SH:(g + 1) * SH])
                for kk in range(SH // 128):
                    kt = g * (SH // 128) + kk
                    ps = pipe.tile([128, NHID], F32, tag="pipe")
                    nc.tensor.matmul(
                        ps[:], xt0[:, kk * 128:(kk + 1) * 128],
                        w1_sb[:, 0:NHID], start=True, stop=False,
                    )
                    nc.tensor.matmul(
                        ps[:], xt1[:, kk * 128:(kk + 1) * 128],
                        w1_sb[:, NHID:2 * NHID], start=False, stop=True,
                    )
                    nc.vector.tensor_copy(s1_sb[:, kt * NHID:(kt + 1) * NHID], ps[:])

            # ---- phase B: h1T = relu( sum_k S1[k,:]^T adjT[k,:] ) ----
            # one PSUM bank per accumulation slice (start=True clears whole bank)
            hps = [
                pacc.tile([NHID, CH], F32, tag="acc", name=f"hps{c}")
                for c in range(NCH)
            ]
            adj_res = []
            for kt in range(KT):
                if kt < RES:
                    # resident: loaded once here, reused by layer 2
                    at = arp.tile([128, SH], F16, tag=f"ar{kt}", name=f"ar{kt}")
                    nc.sync.dma_start(at[:], adjT[kt * 128:(kt + 1) * 128, :])
                    adj_res.append(at)
                else:
                    at = asp.tile([128, SH], F16, tag="adj")
                    nc.sync.dma_start(at[:], adjT[kt * 128:(kt + 1) * 128, :])
                for c in range(NCH):
                    nc.tensor.matmul(
                        hps[c][:],
                        s1_sb[:, kt * NHID:(kt + 1) * NHID],
                        at[:, c * CH:(c + 1) * CH],
                        start=(kt == 0),
                        stop=(kt == KT - 1),
                    )
            for c in range(NCH):
                nc.scalar.activation(h1t_sb[:, c * CH:(c + 1) * CH], hps[c][:], Relu)

            # ---- phase C: S2_local = h1 @ [W2|W3], AllGather ----
            s2l_sb = pp.tile([128, (SH // 128) * 128], F16, tag="s2l")
            for kk in range(SH // 128):
                ps = pipe.tile([128, 128], F32, tag="pipe")
                nc.tensor.matmul(
                    ps[:], h1t_sb[:, kk * 128:(kk + 1) * 128], w23_sb[:],
                    start=True, stop=True,
                )
                nc.vector.tensor_copy(s2l_sb[:, kk * 128:(kk + 1) * 128], ps[:])
            s2in = dp.tile([128, SH], F16, tag="s2in")
            nc.gpsimd.dma_start(s2in[:], s2l_sb[:])
            s2g = dp.tile([NC * 128, SH], F16, tag="s2g", addr_space="Shared")
            nc.gpsimd.collective_compute(
                "AllGather", mybir.AluOpType.bypass,
                replica_groups=rg, ins=[s2in.opt()], outs=[s2g.opt()],
            )
            for g in range(NC):
                nc.gpsimd.dma_start(
                    s2_sb[:, g * SH:(g + 1) * SH],
                    s2g[g * 128:(g + 1) * 128, :],
                )

            # ---- phase D: [muT; logvarT] = sum_k S2[k,:]^T adjT[k,:] ----
            mlps = [
                pacc.tile([128, CH], F32, tag="acc", name=f"mlps{c}")
                for c in range(NCH)
            ]
            for kt in range(KT):
                if kt < RES:
                    at = adj_res[kt]
                else:
                    at = asp.tile([128, SH], F16, tag="adj")
                    nc.sync.dma_start(at[:], adjT[kt * 128:(kt + 1) * 128, :])
                for c in range(NCH):
                    nc.tensor.matmul(
                        mlps[c][:],
                        s2_sb[:, kt * 128:(kt + 1) * 128],
                        at[:, c * CH:(c + 1) * CH],
                        start=(kt == 0),
                        stop=(kt == KT - 1),
                    )
            for c in range(NCH):
                sl = slice(c * CH, (c + 1) * CH)
                nc.scalar.activation(
                    zt_sb[:, sl], mlps[c][0:NHID, :], Copy, scale=1.0 / ZSCALE
                )
            # zT AllGather first (critical path to recon) ...
            zin = dp.tile([NHID, SH], F16, tag="zin")
            nc.gpsimd.dma_start(zin[:], zt_sb[:])
            zg = dp.tile([NC * NHID, SH], F16, tag="zg", addr_space="Shared")
            nc.gpsimd.collective_compute(
                "AllGather", mybir.AluOpType.bypass,
                replica_groups=rg, ins=[zin.opt()], outs=[zg.opt()],
            )
            for g in range(NC):
                nc.gpsimd.dma_start(
                    ztf_sb[:, g * SH:(g + 1) * SH], zg[g * NHID:(g + 1) * NHID, :]
                )
            # ... mu/logvar outputs overlap with the AllGather
            for c in range(NCH):
                sl = slice(c * CH, (c + 1) * CH)
                nc.vector.tensor_copy(mut_sb[:, sl], mlps[c][0:NHID, :])
                nc.scalar.activation(lvt_sb[:, sl], mlps[c][NHID:128, :], Copy)
            nc.gpsimd.dma_start(muT_out[:], mut_sb[:])
            nc.gpsimd.dma_start(lvT_out[:], lvt_sb[:])

            # ---- phase E: recon rows = 256 * (zT_loc.T @ zT_full) ----
            NGR = 4                       # psum chunks per staging tile
            WID = NGR * CH                # 2048 cols per output DMA
            for it in range(SH // 128):
                for cg in range(N // WID):
                    rec = rsp.tile([128, WID], F32, tag="rec")
                    for cc in range(NGR):
                        c = cg * NGR + cc
                        rp = pipe.tile([128, CH], F32, tag="pipe")
                        nc.tensor.matmul(
                            rp[:],
                            zt_sb[:, it * 128:(it + 1) * 128],
                            ztf_sb[:, c * CH:(c + 1) * CH],
                            start=True,
                            stop=True,
                        )
                        sl = slice(cc * CH, (cc + 1) * CH)
                        if cc % 2 == 0:
                            nc.vector.tensor_scalar_mul(
                                rec[:, sl], rp[:], ZSCALE * ZSCALE
                            )
                        else:
                            nc.scalar.activation(
                                rec[:, sl], rp[:], Copy, scale=ZSCALE * ZSCALE
                            )
                    nc.gpsimd.dma_start(
                        recon_out[it * 128:(it + 1) * 128, cg * WID:(cg + 1) * WID],
                        rec[:],
                    )

    nc.compile()
    return nc


def _get_nc():
    if "nc" not in _cache:
        _cache["nc"] = _build_nc()
    return _cache["nc"]


def run_sharded(x, adj, W1, W2, W3, trace=False, tmpdir=None):
    from concourse.bass_utils import run_bass_kernel_spmd

    nc = _get_nc()
    xT16 = np.ascontiguousarray(x.T).astype(np.float16)
    W116 = W1.astype(np.float16)
    W23 = np.concatenate([W2, W3], axis=1).astype(np.float16)
    in_maps = []
    for d in range(NC):
        in_maps.append(
            {
                "adjT": np.ascontiguousarray(adj[d * SH:(d + 1) * SH, :].T).astype(
                    np.float16
                ),
                "xT": xT16,
                "W1": W116,
                "W23": W23,
            }
        )
    return run_bass_kernel_spmd(
        nc, in_maps, list(range(NC)), trace=trace, tmpdir=tmpdir
    )


def kernel(x, adj, W1, W2, W3):
    br = run_sharded(x, adj, W1, W2, W3)
    recon = np.concatenate([br.results[d]["recon_rows"] for d in range(NC)], axis=0)
    mu = np.concatenate([br.results[d]["muT_part"].T for d in range(NC)], axis=0)
    logvar = np.concatenate(
        [br.results[d]["logvarT_part"].T for d in range(NC)], axis=0
    )
    return recon, mu, logvar
